# revision 1
# baseline (speedup 1.0000x reference)
"""BipartiteGraphConvolution on 8 TRN2 NeuronCores (Bass).

Strategy: shard right nodes (and their edges) across the 8 cores. Per core:
  - edges sorted by right node, grouped into 98 groups of 128 right nodes,
    padded to a uniform number of 128-edge tiles per group.
  - device builds L' = left@W_left + b_left as a DRAM table (permuted row
    order for DMA-friendly stores), with a -1e9 dummy row for pad edges.
  - per 128-edge tile: indirect-DMA gather of L' rows; one-hot S built on
    DVE (iota/is_equal); right-feature expansion and per-group aggregation
    of relu(pre) as PE matmuls accumulating in PSUM.
  - per group: W_final/deg*b_final, post/output MLPs as W-stationary
    matmuls in [feat, node] transposed layout; transpose back; DMA out.
No collectives: each core owns its right-node range end to end.
"""
import numpy as np

EMB = 16
N_LEFT = 100000
N_RIGHT = 100000
N_CORES = 8
RPC = 12544            # right nodes per core (98 * 128)
G = RPC // 128         # 98 groups per core
P = 128
LCHUNK = 8192          # Ltab build chunk (columns of left_T)
NCHUNK = 13            # 13 * 8192 = 106496 >= 100000
LT_COLS = NCHUNK * LCHUNK
DUMMY_ROW = LT_COLS    # rows [LT_COLS, LT_COLS+128) = -1e9
LTAB_ROWS = LT_COLS + P
JC = LCHUNK // P       # 64 subchunks per chunk


def _perm_row(n):
    """Table row index for node n under the DMA-friendly store layout."""
    c, r = n // LCHUNK, n % LCHUNK
    j, p = r // P, r % P
    return c * LCHUNK + p * JC + j


def _host_prep(edge_indices, edge_features, right_features):
    l_idx = np.asarray(edge_indices[0], dtype=np.int64)
    r_idx = np.asarray(edge_indices[1], dtype=np.int64)
    f = np.asarray(edge_features, dtype=np.float32).reshape(-1)

    cores = []
    tg_needed = 1
    for c in range(N_CORES):
        base = c * RPC
        m = (r_idx >= base) & (r_idx < base + RPC)
        lc, rc, fc = l_idx[m], r_idx[m] - base, f[m]
        order = np.argsort(rc, kind="stable")
        lc, rc, fc = lc[order], rc[order], fc[order]
        grp = rc // P
        counts = np.bincount(grp, minlength=G)
        tg_needed = max(tg_needed, int(np.ceil(counts.max() / P)))
        cores.append((lc, rc, fc, counts, base))

    TG = tg_needed
    T = G * TG
    EP = T * P

    per_core = []
    for (lc, rc, fc, counts, base) in cores:
        lidx_pad = np.full(EP, DUMMY_ROW, dtype=np.int32)
        qloc_pad = np.zeros(EP, dtype=np.float32)
        f_pad = np.zeros(EP, dtype=np.float32)
        starts = np.concatenate([[0], np.cumsum(counts)]).astype(np.int64)
        perm_l = _perm_row(lc).astype(np.int32)
        for g in range(G):
            s, e = int(starts[g]), int(starts[g + 1])
            n = e - s
            o = g * TG * P
            lidx_pad[o:o + n] = perm_l[s:e]
            qloc_pad[o:o + n] = (rc[s:e] - g * P).astype(np.float32)
            f_pad[o:o + n] = fc[s:e]
        deg = np.bincount(rc, minlength=RPC).astype(np.float32)

        def wrap(a):
            return np.ascontiguousarray(a.reshape(T, P).T)

        rt = np.zeros((EMB, RPC), dtype=np.float32)
        hi = min(base + RPC, N_RIGHT)
        rt[:, :hi - base] = np.asarray(right_features[base:hi],
                                       dtype=np.float32).T
        per_core.append(dict(
            lidx_w=wrap(lidx_pad), qloc_w=wrap(qloc_pad), f_w=wrap(f_pad),
            deg=deg.reshape(1, RPC), right_T=rt,
        ))
    return per_core, TG, T


def _build_nc(TG, T):
    import concourse.bass as bass
    import concourse.mybir as mybir
    from concourse.masks import make_identity

    F32 = mybir.dt.float32
    BF16 = mybir.dt.bfloat16
    AF = mybir.ActivationFunctionType
    OP = mybir.AluOpType

    nc = bass.Bass()
    d_leftT = nc.dram_tensor("left_T", [EMB, LT_COLS], F32, kind="ExternalInput")
    d_rightT = nc.dram_tensor("right_T", [EMB, RPC], F32, kind="ExternalInput")
    d_lidx = nc.dram_tensor("lidx_w", [P, T], mybir.dt.int32, kind="ExternalInput")
    d_qloc = nc.dram_tensor("qloc_w", [P, T], F32, kind="ExternalInput")
    d_f = nc.dram_tensor("f_w", [P, T], F32, kind="ExternalInput")
    d_deg = nc.dram_tensor("deg", [1, RPC], F32, kind="ExternalInput")
    d_wle = nc.dram_tensor("W_left_ext", [EMB + 1, EMB], F32, kind="ExternalInput")
    d_wr = nc.dram_tensor("W_right", [EMB, EMB], F32, kind="ExternalInput")
    d_wvec = nc.dram_tensor("wvec", [1, EMB], F32, kind="ExternalInput")
    d_wf = nc.dram_tensor("W_final", [EMB, EMB], F32, kind="ExternalInput")
    d_bf = nc.dram_tensor("b_final_row", [1, EMB], F32, kind="ExternalInput")
    d_wp = nc.dram_tensor("W_post", [EMB, EMB], F32, kind="ExternalInput")
    d_bp = nc.dram_tensor("b_post", [EMB, 1], F32, kind="ExternalInput")
    d_w1a = nc.dram_tensor("W_out1a", [EMB, EMB], F32, kind="ExternalInput")
    d_w1b = nc.dram_tensor("W_out1b", [EMB, EMB], F32, kind="ExternalInput")
    d_b1 = nc.dram_tensor("b_out1", [EMB, 1], F32, kind="ExternalInput")
    d_w2 = nc.dram_tensor("W_out2", [EMB, EMB], F32, kind="ExternalInput")
    d_b2 = nc.dram_tensor("b_out2", [EMB, 1], F32, kind="ExternalInput")
    d_ones = nc.dram_tensor("ones_row", [1, LCHUNK], F32, kind="ExternalInput")
    d_out = nc.dram_tensor("out", [RPC, EMB], F32, kind="ExternalOutput")
    d_ltab = nc.dram_tensor("ltab", [LTAB_ROWS, EMB], F32, kind="Internal")

    from contextlib import ExitStack
    ctx = ExitStack()
    with ctx:
        identity = ctx.enter_context(nc.sbuf_tensor("identity", [P, P], BF16))
        identity_f = ctx.enter_context(nc.sbuf_tensor("identity_f", [P, P], F32))
        iota_row = ctx.enter_context(nc.sbuf_tensor("iota_row", [P, P], F32))
        wb = ctx.enter_context(nc.sbuf_tensor("wb", [P, EMB], F32))
        rtT = ctx.enter_context(nc.sbuf_tensor("rtT", [EMB, RPC], F32))
        deg_sb = ctx.enter_context(nc.sbuf_tensor("deg_sb", [1, RPC], F32))
        lidx_sb = ctx.enter_context(nc.sbuf_tensor("lidx_sb", [P, T], mybir.dt.int32))
        qloc_sb = ctx.enter_context(nc.sbuf_tensor("qloc_sb", [P, T], F32))
        f_sb = ctx.enter_context(nc.sbuf_tensor("f_sb", [P, T], F32))
        wle_sb = ctx.enter_context(nc.sbuf_tensor("wle_sb", [EMB + 1, EMB], F32))
        wr_sb = ctx.enter_context(nc.sbuf_tensor("wr_sb", [EMB, EMB], F32))
        wvec_sb = ctx.enter_context(nc.sbuf_tensor("wvec_sb", [1, EMB], F32))
        ones_col = ctx.enter_context(nc.sbuf_tensor("ones_col", [1, P], F32))
        wf_sb = ctx.enter_context(nc.sbuf_tensor("wf_sb", [EMB, EMB], F32))
        bf_sb = ctx.enter_context(nc.sbuf_tensor("bf_sb", [1, EMB], F32))
        wp_sb = ctx.enter_context(nc.sbuf_tensor("wp_sb", [EMB, EMB], F32))
        bp_sb = ctx.enter_context(nc.sbuf_tensor("bp_sb", [EMB, 1], F32))
        w1a_sb = ctx.enter_context(nc.sbuf_tensor("w1a_sb", [EMB, EMB], F32))
        w1b_sb = ctx.enter_context(nc.sbuf_tensor("w1b_sb", [EMB, EMB], F32))
        b1_sb = ctx.enter_context(nc.sbuf_tensor("b1_sb", [EMB, 1], F32))
        w2_sb = ctx.enter_context(nc.sbuf_tensor("w2_sb", [EMB, EMB], F32))
        b2_sb = ctx.enter_context(nc.sbuf_tensor("b2_sb", [EMB, 1], F32))
        lchunk = ctx.enter_context(nc.sbuf_tensor("lchunk", [EMB + 1, LCHUNK], F32))
        lstore = ctx.enter_context(nc.sbuf_tensor("lstore", [P, JC * EMB], F32))
        dummy_sb = ctx.enter_context(nc.sbuf_tensor("dummy_sb", [P, EMB], F32))
        gl = ctx.enter_context(nc.sbuf_tensor("gl", [P, 4 * EMB], F32))
        S = ctx.enter_context(nc.sbuf_tensor("S", [P, 2 * P], BF16))
        ST = ctx.enter_context(nc.sbuf_tensor("ST", [P, 2 * P], BF16))
        pre = ctx.enter_context(nc.sbuf_tensor("pre", [P, 2 * EMB], F32))
        pre2 = ctx.enter_context(nc.sbuf_tensor("pre2", [P, 2 * EMB], F32))
        msg = ctx.enter_context(nc.sbuf_tensor("msg", [P, 2 * EMB], BF16))
        rg_sb = ctx.enter_context(nc.sbuf_tensor("rg_sb", [P, 2 * EMB], BF16))
        agg_sb = ctx.enter_context(nc.sbuf_tensor("agg_sb", [P, 2 * EMB], F32))
        tT_sb = ctx.enter_context(nc.sbuf_tensor("tT_sb", [EMB, 2 * P], F32))
        rf_sb = ctx.enter_context(nc.sbuf_tensor("rf_sb", [EMB, 2 * P], F32))
        post_sb = ctx.enter_context(nc.sbuf_tensor("post_sb", [EMB, 2 * P], F32))
        h_sb = ctx.enter_context(nc.sbuf_tensor("h_sb", [EMB, 2 * P], F32))
        o_sb = ctx.enter_context(nc.sbuf_tensor("o_sb", [EMB, 2 * P], F32))
        ot_sb = ctx.enter_context(nc.sbuf_tensor("ot_sb", [P, 2 * EMB], F32))
        bank0 = ctx.enter_context(nc.psum_tensor("bank0", [P, 192], F32))
        exp_ps = ctx.enter_context(nc.psum_tensor("exp_ps", [P, 2 * EMB], F32))
        agg_ps = ctx.enter_context(nc.psum_tensor("agg_ps", [P, 2 * EMB], F32))
        bank3 = ctx.enter_context(nc.psum_tensor("bank3", [P, 2 * P + 2 * EMB], F32))
        fin_ps = ctx.enter_context(nc.psum_tensor("fin_ps", [EMB, 2 * P], F32))
        post_ps = ctx.enter_context(nc.psum_tensor("post_ps", [EMB, 2 * P], F32))
        h_ps = ctx.enter_context(nc.psum_tensor("h_ps", [EMB, 2 * P], F32))
        o_ps = ctx.enter_context(nc.psum_tensor("o_ps", [EMB, 2 * P], F32))
        s_init = ctx.enter_context(nc.semaphore("s_init"))
        s_ginit = ctx.enter_context(nc.semaphore("s_ginit"))
        s_lt_in = ctx.enter_context(nc.semaphore("s_lt_in"))
        s_lt_mm = ctx.enter_context(nc.semaphore("s_lt_mm"))
        s_lt_cp = ctx.enter_context(nc.semaphore("s_lt_cp"))
        s_lt_out = ctx.enter_context(nc.semaphore("s_lt_out"))
        s_dummy = ctx.enter_context(nc.semaphore("s_dummy"))
        s_wb = ctx.enter_context(nc.semaphore("s_wb"))
        s_gl = ctx.enter_context(nc.semaphore("s_gl"))
        s_S = ctx.enter_context(nc.semaphore("s_S"))
        s_tr = ctx.enter_context(nc.semaphore("s_tr"))
        s_stc = ctx.enter_context(nc.semaphore("s_stc"))
        s_exp = ctx.enter_context(nc.semaphore("s_exp"))
        s_pre = ctx.enter_context(nc.semaphore("s_pre"))
        s_msg = ctx.enter_context(nc.semaphore("s_msg"))
        s_agg = ctx.enter_context(nc.semaphore("s_agg"))
        s_rgp = ctx.enter_context(nc.semaphore("s_rgp"))
        s_rgc = ctx.enter_context(nc.semaphore("s_rgc"))
        s_aggc = ctx.enter_context(nc.semaphore("s_aggc"))
        s_ttr = ctx.enter_context(nc.semaphore("s_ttr"))
        s_ttc = ctx.enter_context(nc.semaphore("s_ttc"))
        s_fin = ctx.enter_context(nc.semaphore("s_fin"))
        s_rf = ctx.enter_context(nc.semaphore("s_rf"))
        s_pp = ctx.enter_context(nc.semaphore("s_pp"))
        s_pc = ctx.enter_context(nc.semaphore("s_pc"))
        s_hp = ctx.enter_context(nc.semaphore("s_hp"))
        s_hc = ctx.enter_context(nc.semaphore("s_hc"))
        s_op = ctx.enter_context(nc.semaphore("s_op"))
        s_oc = ctx.enter_context(nc.semaphore("s_oc"))
        s_otp = ctx.enter_context(nc.semaphore("s_otp"))
        s_otc = ctx.enter_context(nc.semaphore("s_otc"))
        s_out = ctx.enter_context(nc.semaphore("s_out"))
        block = ctx.enter_context(nc.Block())
        NLOAD = 19 * 16  # 17 preamble dmas, inc 16 each

        def e16(x):  # column slice helpers
            return slice(x * EMB, (x + 1) * EMB)

        def e128(x):
            return slice(x * P, (x + 1) * P)

        @block.sync
        def _(sync):
            for src, dst in [
                (d_rightT[:, :], rtT[:]), (d_deg[:, :], deg_sb[:]),
                (d_lidx[:, :], lidx_sb[:]), (d_qloc[:, :], qloc_sb[:]),
                (d_f[:, :], f_sb[:]), (d_wle[:, :], wle_sb[:]),
                (d_wr[:, :], wr_sb[:]), (d_wvec[:, :], wvec_sb[:]),
                (d_wf[:, :], wf_sb[:]), (d_bf[:, :], bf_sb[:]),
                (d_wp[:, :], wp_sb[:]), (d_bp[:, :], bp_sb[:]),
                (d_w1a[:, :], w1a_sb[:]), (d_w1b[:, :], w1b_sb[:]),
                (d_b1[:, :], b1_sb[:]), (d_w2[:, :], w2_sb[:]),
                (d_b2[:, :], b2_sb[:]),
                (d_ones[:, :], lchunk[EMB:EMB + 1, :]),
                (d_ones[:, :P], ones_col[:]),
            ]:
                sync.dma_start(out=dst, in_=src).then_inc(s_init, 16)
            for c in range(NCHUNK):
                if c >= 1:
                    sync.wait_ge(s_lt_mm, c * JC)  # PE done with prev chunk
                sync.dma_start(
                    out=lchunk[:EMB, :],
                    in_=d_leftT[:, c * LCHUNK:(c + 1) * LCHUNK],
                ).then_inc(s_lt_in, 16)
                sync.wait_ge(s_lt_cp, (c + 1) * JC)
                sync.dma_start(
                    out=d_ltab[c * LCHUNK:(c + 1) * LCHUNK, :].rearrange(
                        "(p j) d -> p j d", p=P),
                    in_=lstore[:].rearrange("p (j d) -> p j d", d=EMB),
                ).then_inc(s_lt_out, 16)
            sync.wait_ge(s_dummy, 1)
            sync.dma_start(out=d_ltab[LT_COLS:LT_COLS + P, :],
                           in_=dummy_sb[:]).then_inc(s_lt_out, 16)
            for g in range(G):
                sync.wait_ge(s_otc, g + 1)
                sync.dma_start(out=d_out[e128(g), :],
                               in_=ot_sb[:, e16(g % 2)]).then_inc(s_out, 16)

        @block.gpsimd
        def _(gpsimd):
            gpsimd.memset(identity[:], 0.0)
            make_identity(nc, identity[:], nomemset=True)
            gpsimd.memset(identity_f[:], 0.0)
            make_identity(nc, identity_f[:], nomemset=True)
            gpsimd.iota(iota_row[:], [[1, P]], channel_multiplier=0,
                        allow_small_or_imprecise_dtypes=True)
            gpsimd.memset(dummy_sb[:], -1e9).then_inc(s_dummy, 1)
            gpsimd.sem_inc(s_ginit, 1)
            gpsimd.wait_ge(s_init, NLOAD)
            gpsimd.wait_ge(s_lt_out, 16 * (NCHUNK + 1))
            for t in range(T):
                if t >= 4:
                    gpsimd.wait_ge(s_pre, t - 3)
                gpsimd.indirect_dma_start(
                    out=gl[:, e16(t % 4)],
                    out_offset=None,
                    in_=d_ltab[:, :],
                    in_offset=bass.IndirectOffsetOnAxis(
                        ap=lidx_sb[:, t:t + 1], axis=0),
                ).then_inc(s_gl, 16)

        @block.tensor
        def _(tensor):
            tensor.wait_ge(s_init, NLOAD)
            tensor.wait_ge(s_ginit, 1)
            for c in range(NCHUNK):
                tensor.wait_ge(s_lt_in, 16 * (c + 1))
                for j in range(JC):
                    it = c * JC + j
                    if it >= 1:
                        tensor.wait_ge(s_lt_cp, it)
                    tensor.matmul(
                        bank0[:, 160:176], lhsT=lchunk[:, e128(j)], rhs=wle_sb[:],
                        start=True, stop=True,
                    ).then_inc(s_lt_mm, 1)
            tensor.wait_ge(s_lt_cp, NCHUNK * JC)
            tensor.matmul(bank0[:, 176:192], lhsT=ones_col[:],
                          rhs=wvec_sb[:],
                          start=True, stop=True).then_inc(s_wb, 1)
            for g in range(G):
                if g >= 2:
                    tensor.wait_ge(s_rgc, g - 1)
                if g >= 1:
                    tensor.wait_ge(s_stc, g * TG)
                tensor.matmul(bank0[:, 128 + (g % 2) * EMB:128 + (g % 2) * EMB + EMB],
                              lhsT=rtT[:, e128(g)], rhs=wr_sb[:],
                              start=True, stop=True).then_inc(s_rgp, 1)
                for k in range(TG):
                    t = g * TG + k
                    tensor.wait_ge(s_S, t + 1)
                    if t >= 2:
                        tensor.wait_ge(s_stc, t - 1)
                    tensor.transpose(bank0[:, (t % 2) * 64:(t % 2) * 64 + 64].bitcast(BF16), S[:, e128(t % 2)],
                                     identity[:]).then_inc(s_tr, 1)
                    tensor.wait_ge(s_stc, t + 1)
                    tensor.wait_ge(s_rgc, g + 1)
                    if t >= 1:
                        tensor.wait_ge(s_pre, t)
                    tensor.matmul(exp_ps[:, e16(t % 2)],
                                  lhsT=ST[:, e128(t % 2)],
                                  rhs=rg_sb[:, e16(g % 2)],
                                  start=True, stop=True).then_inc(s_exp, 1)
                    tensor.wait_ge(s_msg, t + 1)
                    if k == 0 and g >= 1:
                        tensor.wait_ge(s_aggc, g)
                    tensor.matmul(agg_ps[:, e16(g % 2)],
                                  lhsT=S[:, e128(t % 2)],
                                  rhs=msg[:, e16(t % 2)],
                                  start=(k == 0), stop=(k == TG - 1),
                                  skip_group_check=True).then_inc(s_agg, 1)
                tensor.wait_ge(s_aggc, g + 1)
                if g >= 2:
                    tensor.wait_ge(s_ttc, g - 1)
                if g >= 1:
                    tensor.wait_ge(s_otc, g)
                tensor.transpose(bank3[:EMB, e128(g % 2)], agg_sb[:, e16(g % 2)],
                                 identity_f[:]).then_inc(s_ttr, 1)
                tensor.wait_ge(s_ttc, g + 1)
                if g >= 2:
                    tensor.wait_ge(s_rf, g - 1)
                tensor.matmul(fin_ps[:, e128(g % 2)], lhsT=wf_sb[:],
                              rhs=tT_sb[:, e128(g % 2)], start=True,
                              stop=False, skip_group_check=True)
                tensor.matmul(fin_ps[:, e128(g % 2)], lhsT=bf_sb[:],
                              rhs=deg_sb[:, e128(g)], start=False,
                              stop=True, skip_group_check=True).then_inc(s_fin, 1)
                tensor.wait_ge(s_rf, g + 1)
                if g >= 2:
                    tensor.wait_ge(s_pc, g - 1)
                tensor.matmul(post_ps[:, e128(g % 2)], lhsT=wp_sb[:],
                              rhs=rf_sb[:, e128(g % 2)], start=True,
                              stop=True).then_inc(s_pp, 1)
                tensor.wait_ge(s_pc, g + 1)
                if g >= 2:
                    tensor.wait_ge(s_hc, g - 1)
                tensor.matmul(h_ps[:, e128(g % 2)], lhsT=w1a_sb[:],
                              rhs=post_sb[:, e128(g % 2)], start=True,
                              stop=False, skip_group_check=True)
                tensor.matmul(h_ps[:, e128(g % 2)], lhsT=w1b_sb[:],
                              rhs=rtT[:, e128(g)], start=False,
                              stop=True, skip_group_check=True).then_inc(s_hp, 1)
                tensor.wait_ge(s_hc, g + 1)
                if g >= 2:
                    tensor.wait_ge(s_oc, g - 1)
                tensor.matmul(o_ps[:, e128(g % 2)], lhsT=w2_sb[:],
                              rhs=h_sb[:, e128(g % 2)], start=True,
                              stop=True).then_inc(s_op, 1)
                tensor.wait_ge(s_oc, g + 1)
                if g >= 2:
                    tensor.wait_ge(s_otc, g - 1)
                tensor.transpose(bank3[:, 2 * P + (g % 2) * EMB:2 * P + (g % 2) * EMB + EMB], o_sb[:, e128(g % 2)],
                                 identity_f[:EMB, :EMB]).then_inc(s_otp, 1)

        @block.vector
        def _(vector):
            vector.wait_ge(s_init, NLOAD)
            vector.wait_ge(s_ginit, 1)
            for c in range(NCHUNK):
                for j in range(JC):
                    it = c * JC + j
                    vector.wait_ge(s_lt_mm, it + 1)
                    if c >= 1 and j == 0:
                        vector.wait_ge(s_lt_out, 16 * c)  # lstore freed
                    vector.tensor_copy(out=lstore[:, e16(j)],
                                       in_=bank0[:, 160:176]).then_inc(s_lt_cp, 1)
            vector.wait_ge(s_wb, 1)
            vector.tensor_copy(out=wb[:], in_=bank0[:, 176:192])
            for g in range(G):
                vector.wait_ge(s_rgp, g + 1)
                if g >= 2:
                    vector.wait_ge(s_exp, (g - 1) * TG)
                vector.tensor_copy(out=rg_sb[:, e16(g % 2)],
                                   in_=bank0[:, 128 + (g % 2) * EMB:128 + (g % 2) * EMB + EMB]).then_inc(s_rgc, 1)
                for k in range(TG):
                    t = g * TG + k
                    if t >= 2:
                        vector.wait_ge(s_agg, t - 1)
                    vector.tensor_tensor(
                        out=S[:, e128(t % 2)],
                        in0=qloc_sb[:, t:t + 1].to_broadcast([P, P]),
                        in1=iota_row[:],
                        op=OP.is_equal,
                    ).then_inc(s_S, 1)
                    vector.wait_ge(s_tr, t + 1)
                    if t >= 2:
                        vector.wait_ge(s_exp, t - 1)
                    vector.tensor_copy(out=ST[:, e128(t % 2)],
                                       in_=bank0[:, (t % 2) * 64:(t % 2) * 64 + 64].bitcast(BF16)
                                       ).then_inc(s_stc, 1)
                    vector.wait_ge(s_gl, 16 * (t + 1))
                    vector.wait_ge(s_exp, t + 1)
                    vector.tensor_tensor(out=pre[:, e16(t % 2)],
                                         in0=gl[:, e16(t % 4)],
                                         in1=exp_ps[:, e16(t % 2)], op=OP.add)
                    if t >= 2:
                        vector.wait_ge(s_msg, t - 1)
                    vector.scalar_tensor_tensor(
                        out=pre2[:, e16(t % 2)], in0=wb[:],
                        scalar=f_sb[:, t:t + 1], in1=pre[:, e16(t % 2)],
                        op0=OP.mult, op1=OP.add,
                    ).then_inc(s_pre, 1)
                vector.wait_ge(s_agg, (g + 1) * TG)
                if g >= 2:
                    vector.wait_ge(s_ttr, g - 1)
                vector.tensor_copy(out=agg_sb[:, e16(g % 2)],
                                   in_=agg_ps[:, e16(g % 2)]).then_inc(s_aggc, 1)
                vector.wait_ge(s_ttr, g + 1)
                if g >= 2:
                    vector.wait_ge(s_fin, g - 1)
                vector.tensor_copy(out=tT_sb[:, e128(g % 2)],
                                   in_=bank3[:EMB, e128(g % 2)]).then_inc(s_ttc, 1)
                vector.wait_ge(s_otp, g + 1)
                if g >= 2:
                    vector.wait_ge(s_out, 16 * (g - 1))
                vector.tensor_copy(out=ot_sb[:, e16(g % 2)],
                                   in_=bank3[:, 2 * P + (g % 2) * EMB:2 * P + (g % 2) * EMB + EMB]).then_inc(s_otc, 1)

        @block.scalar
        def _(scalar):
            scalar.wait_ge(s_init, NLOAD)
            scalar.wait_ge(s_ginit, 1)
            for g in range(G):
                for k in range(TG):
                    t = g * TG + k
                    scalar.wait_ge(s_pre, t + 1)
                    if t >= 2:
                        scalar.wait_ge(s_agg, t - 1)
                    scalar.activation(out=msg[:, e16(t % 2)],
                                      in_=pre2[:, e16(t % 2)],
                                      func=AF.Relu).then_inc(s_msg, 1)
                scalar.wait_ge(s_fin, g + 1)
                if g >= 2:
                    scalar.wait_ge(s_pp, g - 1)
                scalar.activation(out=rf_sb[:, e128(g % 2)],
                                  in_=fin_ps[:, e128(g % 2)],
                                  func=AF.Relu).then_inc(s_rf, 1)
                scalar.wait_ge(s_pp, g + 1)
                if g >= 2:
                    scalar.wait_ge(s_hp, g - 1)
                scalar.activation(out=post_sb[:, e128(g % 2)],
                                  in_=post_ps[:, e128(g % 2)],
                                  func=AF.Identity,
                                  bias=bp_sb[:, :]).then_inc(s_pc, 1)
                scalar.wait_ge(s_hp, g + 1)
                if g >= 2:
                    scalar.wait_ge(s_op, g - 1)
                scalar.activation(out=h_sb[:, e128(g % 2)],
                                  in_=h_ps[:, e128(g % 2)],
                                  func=AF.Relu,
                                  bias=b1_sb[:, :]).then_inc(s_hc, 1)
                scalar.wait_ge(s_op, g + 1)
                if g >= 2:
                    scalar.wait_ge(s_otp, g - 1)
                scalar.activation(out=o_sb[:, e128(g % 2)],
                                  in_=o_ps[:, e128(g % 2)],
                                  func=AF.Identity,
                                  bias=b2_sb[:, :]).then_inc(s_oc, 1)

    return nc


def kernel(left_features, edge_indices, edge_features, right_features,
           W_left, b_left, W_edge, W_right, W_final, b_final,
           W_post, b_post, W_out1, b_out1, W_out2, b_out2):
    from concourse.bass_utils import run_bass_kernel_spmd

    left_features = np.asarray(left_features, dtype=np.float32)
    right_features = np.asarray(right_features, dtype=np.float32)

    per_core, TG, T = _host_prep(edge_indices, edge_features, right_features)
    nc = _build_nc(TG, T)

    leftT = np.zeros((EMB, LT_COLS), dtype=np.float32)
    leftT[:, :N_LEFT] = left_features.T
    wle = np.vstack([np.asarray(W_left, np.float32),
                     np.asarray(b_left, np.float32).reshape(1, EMB)])
    shared = dict(
        left_T=leftT, W_left_ext=wle,
        ones_row=np.ones((1, LCHUNK), dtype=np.float32),
        W_right=np.asarray(W_right, np.float32),
        wvec=np.asarray(W_edge, np.float32).reshape(1, EMB),
        W_final=np.asarray(W_final, np.float32),
        b_final_row=np.asarray(b_final, np.float32).reshape(1, EMB),
        W_post=np.asarray(W_post, np.float32),
        b_post=np.asarray(b_post, np.float32).reshape(EMB, 1),
        W_out1a=np.ascontiguousarray(np.asarray(W_out1, np.float32)[:EMB, :]),
        W_out1b=np.ascontiguousarray(np.asarray(W_out1, np.float32)[EMB:, :]),
        b_out1=np.asarray(b_out1, np.float32).reshape(EMB, 1),
        W_out2=np.asarray(W_out2, np.float32),
        b_out2=np.asarray(b_out2, np.float32).reshape(EMB, 1),
    )
    in_maps = []
    for c in range(N_CORES):
        m = dict(shared)
        m.update(per_core[c])
        in_maps.append(m)

    res = run_bass_kernel_spmd(nc, in_maps, core_ids=list(range(N_CORES)))
    out = np.concatenate([res.results[c]["out"] for c in range(N_CORES)],
                         axis=0)
    return np.ascontiguousarray(out[:N_RIGHT])

